# revision 84
# baseline (speedup 1.0000x reference)
"""Trainium2 Bass kernel for nn_DotProductAttention_76338748719461.

Attention with a multiplicative mask and softmax over the QUERY axis
(axis=1 of [B, Lq, Lk] scores):

    S[b,q,k]  = (Q[b,q,:] . K[b,k,:]) / 8 + max(log(mask[0,q,k]), F32_MIN)
    A         = softmax(S, axis=q)
    out[b,q,v]= sum_k A[b,q,k] * V[b,k,v]

Key identity: exp(S + log m) = exp(S) * m, so the mask is applied as a
multiply after exp — no log, no additive bias, and mask==0 handled exactly.

Design (per NeuronCore; batch data-parallel over 8 cores, 2 per core):
  * ALL layout work happens on host: Q^T (pre-scaled by 1/8) and K^T in
    f16, V in f32, and the mask TRANSPOSED to [k, q] in f16; output
    returns f16 and is upcast on host. The device does zero transposes
    and zero dtype-conversion DMAs (v1 burned ~33 MB/core of HBM
    traffic and a whole prep phase on mask cast+transpose).
  * Work in the transposed score orientation S_T[k, q], so the softmax
    reduction (over q) is a free-axis reduction.
  * Software pipeline at q-half granularity over 32 (batch, k-tile)
    units: three rotating [128, 1024] PSUM score tiles (6 banks) decouple
    the per-half chain QK (PE, f16) -> exp (ACT, the pacer at ~55us/core
    busy) -> PM = E*mask & row-sum D -> Vp = V/D (Pool normalize_recip)
    -> AV.
  * MIXED mask application (XJ=6): k-tiles j<6 accumulate log(mask) into
    the score PSUM via a PE identity-matmul (exp output IS the masked
    weight; one DVE TS-accum for D); tiles j>=6 use the DVE full-width
    multiply + TS-accum. This balances DVE (~40us) against spare PE
    (~55us) — DVE f16 ops run at 1-2x on real HW, not the model's 2-4x,
    and an all-DVE mask path co-paces with ACT at ~100us/pass.
  * AV with SWAPPED operand roles: stationary = PM chunk [128k, 128q],
    moving = Vp [128k, 64v] => out[q, v] accumulates directly in PSUM
    ([128, 16, 64] f32 = 2 banks, lazy-zero bank groups); no output
    transposes. AV for unit u is emitted at unit u+5 so the in-order PE
    stream never stalls on the denominator chain.
  * ~20 dummy PE transposes at kernel start ride the input-DMA wait to
    finish the 3us p-state ramp before the first real matmul.

Measured (8 cores, in-NEFF repetition differential, paired median):
~81us steady-state per pass (v1 baseline: 139us differential, 207us
harness); scale-relative absmax error 7.9e-4 vs the fp32 reference.
Ablations show the DVE is the HW pacer (~55us busy; exp on ACT hides
completely underneath), so denominators use READ-ONLY tensor_reduce
(not TS-accum, which re-writes the tensor) and the PSUM drain runs on
ACT via activation(Copy).

Hard-won HW facts (cost model/CoreSim do NOT flag these):
  * InstTensorTensorReduce (ISA op) crashes the DVE exec unit on TRN2.
  * activation(accum_out=...) returns wrong sums on HW (sim is fine) —
    compute softmax denominators with a DVE TS-accum instead.
  * TensorScalar/free-axis TensorReduce are illegal opcodes on Pool;
    Pool is the GPSIMD DSP engine (custom ucode ops only, no PSUM).
  * DMAs issued from the ACT queue stall the exp stream badly.
  * gpsimd-queue DMAs burn ~1.3us of Pool ENGINE time each (SWDGE).
"""

import os
import numpy as np

B, LQ, LK, D, DV = 16, 2048, 2048, 64, 64
NCORES = 8
BPC = B // NCORES  # batches per core
P = 128
CH = 512  # QK matmul moving chunk (one PSUM bank of fp32)
NT_Q = LQ // P  # 16
NT_K = LK // P  # 16
SCALE = 1.0 / 8.0  # 1/sqrt(64), folded into host-side Q^T prep

MAIN_REPS = int(os.environ.get("MAIN_REPS", "1"))  # repeat body (timing builds)
ABLATE = os.environ.get("ABLATE", "")  # timing-only ablations: nodve|noact|noav
# k-tiles j < XJ use the "additive" mask method: log(mask) is accumulated
# into the score PSUM by a PE identity-matmul and ACT's exp accumulates the
# softmax denominator itself — zero DVE work for those units. The rest use
# the DVE multiply path. This balances DVE (whose f16 ops run at 1-2x on
# real HW, not the cost model's 2-4x) against the PE's spare throughput.
XJ = int(os.environ.get("XJ", "6"))

_CACHED = None


def prep_core_inputs(query, key, value, mask):
    """Host-side layout prep: per-core input dicts for the device binary.

    qt: [BPC, 64, 2048] f16 = (Q/8)^T     kt: [BPC, 64, 2048] f16 = K^T
    v:  [BPC, 2048, 64] f16               mt: [2048, 2048] f16 = mask[0]^T
    """
    query = np.asarray(query, dtype=np.float32)
    key = np.asarray(key, dtype=np.float32)
    value = np.asarray(value, dtype=np.float32)
    mask = np.asarray(mask, dtype=np.float32)

    q16 = (query.transpose(0, 2, 1) * np.float32(SCALE)).astype(np.float16)
    k16 = key.transpose(0, 2, 1).astype(np.float16)
    mt = mask[0].T  # [k, q]
    m16 = mt[XJ * P :].astype(np.float16)  # multiply-path tiles (j >= XJ)
    with np.errstate(divide="ignore"):
        lm16 = np.log(mt[: XJ * P]).astype(np.float16)  # additive-path tiles
    return [
        {
            "qt": np.ascontiguousarray(q16[c * BPC : (c + 1) * BPC]),
            "kt": np.ascontiguousarray(k16[c * BPC : (c + 1) * BPC]),
            "v": np.ascontiguousarray(value[c * BPC : (c + 1) * BPC]),
            "mt": m16,
            "lm": lm16,
        }
        for c in range(NCORES)
    ]


def _emit_av(nc, O, PM, Vp, j):
    """AV with swapped roles: stationary PM chunk [128k, 128q], moving
    Vp [128k, 64v] -> out[q, v] accumulates directly in PSUM (2 banks).

    PSUM accumulation groups are bank-granular (2 KB zero regions, 8
    chunks of [128, 64] f32 per bank): open each bank's group with
    start=True on its first chunk at j==0 (lazy-zeroes the whole bank;
    later j==0 chunks land on pending-zero bytes and replace), close it
    with stop=True on its last chunk at j==15."""
    from concourse.bass import ds

    for t in range(NT_Q):
        nc.tensor.matmul(
            O[:, t, :],
            PM[:, ds(P * t, P)],
            Vp[:],
            start=(j == 0 and t % 8 == 0),
            stop=(j == NT_K - 1 and t % 8 == 7),
            skip_group_check=True,
        )


def _emit_out(nc, work, o_d, pO, pb, ident, psum_o):
    """Evacuate the [q, v]-oriented PSUM accumulator: DVE copy to SBUF
    (f32 PSUM -> f16, host upcasts), then DMA on the sync queue — in two
    halves so the copy and the DMA pipeline."""
    import concourse.mybir as mybir
    from concourse.bass import ds

    import concourse.mybir as _mb

    out_sb = work.tile(
        [P, NT_Q, DV], mybir.dt.float16, tag="osb", bufs=2, name="out_sb"
    )
    dst = o_d[pb].rearrange("(t p) d -> p t d", p=P)
    half = NT_Q // 2
    for g in range(2):
        gs = ds(g * half, half)
        # evacuate on ACT (it has slack; DVE is the pacer)
        nc.scalar.activation(
            out_sb[:, gs, :], pO[:, gs, :],
            _mb.ActivationFunctionType.Copy,
        )
        nc.sync.dma_start(dst[:, gs, :], out_sb[:, gs, :])


def _emit_pass(nc, tc, pools, aps, dts):
    """One full pass: input DMAs + 2 batches x 16 k-tiles + output DMAs."""
    import concourse.mybir as mybir
    from concourse.bass import ds, ts

    qt_d, kt_d, v_d, mt_d, lm_d, o_d = aps
    f32, f16, AF = dts
    ALU = mybir.AluOpType
    big, psum_s, psum_o, work, small, ident, ident16 = pools
    HF0 = LQ // 2

    mT = big.tile([P, NT_K - XJ, LQ], f16, tag="mT", name="mT")
    lmT = big.tile([P, XJ, LQ], f16, tag="lmT", name="lmT")
    QT = big.tile([D, BPC, LQ], f16, tag="QT", name="QT")
    KT = big.tile([D, BPC, LK], f16, tag="KT", name="KT")
    Vn = big.tile([P, BPC, NT_K, DV], f32, tag="Vn", name="Vn")

    # ALL input DMAs go on the sync queue (hardware DGE — the gpsimd
    # queue's software DGE burns ~1.3us of Pool ENGINE time per
    # transfer), hand-ordered so each tile lands just before its first
    # use: batch 0's K/Q first (first QK), then early mask tiles (tile j
    # is consumed at ~2.2us*j), V0 (first Vp), batch 1's K/Q, and the
    # remaining mask tiles, which stay ahead of consumption from there.
    def dma_v(b):
        nc.sync.dma_start(
            Vn[:, b, :, :], v_d[b].rearrange("(t p) d -> p t d", p=P)
        )

    def dma_m(j):
        # all mask tiles on the SP hardware-DGE queue (DMAs issued from
        # the ACT queue measurably stall the exp stream — do not split)
        if j < XJ:
            nc.sync.dma_start(lmT[:, j, :], lm_d[ds(P * j, P), :])
        else:
            nc.sync.dma_start(
                mT[:, j - XJ, :], mt_d[ds(P * (j - XJ), P), :]
            )

    # just the slices the first QK touches (~150 KB), so the first score
    # matmul can issue ~1us earlier than a full-tile load would allow
    nc.sync.dma_start(KT[:, 0, ds(0, P)], kt_d[0][:, ds(0, P)])
    nc.sync.dma_start(QT[:, 0, ds(0, HF0)], qt_d[0][:, ds(0, HF0)])
    nc.sync.dma_start(KT[:, 0, ds(P, LK - P)], kt_d[0][:, ds(P, LK - P)])
    nc.sync.dma_start(QT[:, 0, ds(HF0, HF0)], qt_d[0][:, ds(HF0, HF0)])
    dma_m(0)
    dma_m(1)
    dma_m(2)
    dma_v(0)
    nc.sync.dma_start(KT[:, 1, :], kt_d[1])
    nc.sync.dma_start(QT[:, 1, :], qt_d[1])
    dma_m(3)
    dma_v(1)
    for j in range(4, NT_K):
        dma_m(j)

    # Software pipeline at q-half granularity. The two q-halves of the
    # score tile live in SEPARATE PSUM tiles (2 banks each) so dependency
    # tracking is per-half: QK for half t+1 is emitted one ACT-slot ahead
    # of ACT for half t, so the exp stream never waits on the PE.
    # Cross-engine consumers are emitted with a lag so no in-order engine
    # stream ever blocks on a slow producer:
    #   - reciprocal/Vp for unit u are emitted during unit u+1,
    #   - AV matmuls for unit u are emitted during unit u+DEFER (the
    #     denominator chain ACT->DVE->Pool->recip->Vp is ~2.5 units long
    #     and the in-order PE stream would stall on the AV Ldweights).
    # Three rotating half-score tiles (2 banks each; the swapped AV's
    # 2-bank O frees the room): QK for half t+1 only has a WAR against
    # ACT of half t-2, giving the PE a full extra ACT slot of slack.
    HF = LQ // 2
    NS = 3
    S3 = [
        psum_s.tile([P, HF], f32, tag=f"s{h}", name=f"S{h}")
        for h in range(NS)
    ]
    h0s, h1s = ds(0, HF), ds(HF, HF)
    DEFER = 5

    # PE p-state warmup: ~20 dep-free dummy transposes into S3[0] (junk;
    # overwritten by the first QK) run during the input-DMA wait so the
    # 3us ramp to full clock is over by the time real matmuls issue.
    for _ in range(20):
        nc.tensor.transpose(
            S3[0][0:DV, 0:DV], ident[0:DV, 0:DV], ident[0:DV, 0:DV]
        )

    units = [(b, j) for b in range(BPC) for j in range(NT_K)]
    NU = len(units)
    ctx = {}  # u -> dict of tiles
    O_of = {}  # b -> O psum tile

    def emit_qk(t):
        u, h = t // 2, t % 2
        b, j = units[u]
        addm = j < XJ  # additive-mask unit: accumulate log(mask) on PE
        for c in range(2):
            nc.tensor.matmul(
                S3[t % NS][:, ts(c, CH)],
                KT[:, b, ds(P * j, P)],
                QT[:, b, ds(HF * h + CH * c, CH)],
                start=True,
                stop=not addm,
            )
        if addm:
            for c in range(2):
                nc.tensor.matmul(
                    S3[t % NS][:, ts(c, CH)],
                    ident16[:],
                    lmT[:, j, ds(HF * h + CH * c, CH)],
                    start=False,
                    stop=True,
                )

    emit_qk(0)
    for t in range(2 * (NU + DEFER) + 2):
        u, h = t // 2, t % 2
        if t + 1 < 2 * NU:
            emit_qk(t + 1)
        if h == 1:
            # Lagged denominator combine for A-unit u-1: the ACT accum_out
            # write lands ASYNCHRONOUSLY after the instruction's completion
            # semaphore (reading it immediately races and returns garbage
            # on HW — measured), so combine one unit later.
            if 0 <= u - 1 < NU and "DA2" in ctx.get(u - 1, {}):
                cp = ctx[u - 1]
                Dp = small.tile([P, 1], f32, tag="d", name="Dp")
                nc.vector.tensor_tensor(
                    Dp[:], cp["DA2"][:, ds(0, 1)], cp["DA2"][:, ds(1, 1)],
                    ALU.add,
                )
                cp["Dsum"] = Dp
            # Vp for unit u-2 (both unit types have Dsum by then)
            if 0 <= u - 2 < NU and "Dsum" in ctx.get(u - 2, {}):
                cp = ctx[u - 2]
                bp, jp = units[u - 2]
                Vp = small.tile([P, DV], f16, tag="vp", name="Vp")
                nc.gpsimd.normalize_recip(
                    Vp[:], Vn[:, bp, jp, :], cp["Dsum"][:]
                )
                cp["Vp"] = Vp
        if h == 1 and u - DEFER >= 0 and u - DEFER < NU:
            ua = u - DEFER
            ca = ctx[ua]
            ba, ja = units[ua]
            if ABLATE != "noav":
                _emit_av(nc, O_of[ba], ca["PM"], ca["Vp"], ja)
            if ja == NT_K - 1:
                _emit_out(nc, work, o_d, O_of[ba], ba, ident, psum_o)
            del ctx[ua]
        if u >= NU:
            continue
        b, j = units[u]
        addm = j < XJ
        if h == 0:
            E = work.tile([P, LQ], f16, tag="e", name="E")
            ctx[u] = {"E": E}
            if addm:
                ctx[u]["DA2"] = small.tile([P, 2], f32, tag="da2", name="DA2")
            else:
                ctx[u]["PM"] = work.tile([P, LQ], f16, tag="pm", name="PM")
            if b not in O_of:
                O_of[b] = psum_o.tile(
                    [P, NT_Q, DV], f32, tag="o", name=f"O{b}"
                )
                if ABLATE == "noav":
                    nc.vector.memset(O_of[b][:, 0, :], 0.0)
            if ABLATE != "noact":
                if addm:
                    nc.scalar.activation(
                        E[:, h0s], S3[t % NS][:], AF.Exp,
                        accum_out=ctx[u]["DA2"][:, ds(0, 1)],
                    )
                else:
                    nc.scalar.activation(E[:, h0s], S3[t % NS][:], AF.Exp)
            else:
                nc.vector.memset(E[:, ds(0, 32)], 1.0)
        else:
            E = ctx[u]["E"]
            if ABLATE != "noact":
                if addm:
                    nc.scalar.activation(
                        E[:, h1s], S3[t % NS][:], AF.Exp,
                        accum_out=ctx[u]["DA2"][:, ds(1, 1)],
                    )
                else:
                    nc.scalar.activation(E[:, h1s], S3[t % NS][:], AF.Exp)
            if addm:
                # additive-mask unit: masked weights ARE exp's output and
                # the denominator halves came from exp's accum_out — zero
                # full-width DVE passes.
                ctx[u]["PM"] = E
            elif ABLATE != "nodve":
                PM = ctx[u]["PM"]
                Dsum = small.tile([P, 1], f32, tag="d", name="Dsum")
                # DVE: full-width mask multiply, then read-only reduction
                nc.vector.tensor_tensor(
                    PM[:], E[:], mT[:, j - XJ, :], ALU.mult
                )
                nc.vector.tensor_reduce(
                    Dsum[:], PM[:], axis=mybir.AxisListType.X, op=ALU.add
                )
                ctx[u]["Dsum"] = Dsum
            else:  # timing ablation: tiny writes so tiles count as allocated
                Dsum = small.tile([P, 1], f32, tag="d", name="Dsum")
                nc.vector.memset(ctx[u]["PM"][:, ds(0, 32)], 1.0)
                nc.vector.memset(Dsum[:], 1.0)
                ctx[u]["Dsum"] = Dsum


def _build_module():
    import concourse.mybir as mybir
    import concourse.tile as tile
    from concourse import bacc
    from contextlib import ExitStack

    f32 = mybir.dt.float32
    f16 = mybir.dt.float16
    dts = (f32, f16, mybir.ActivationFunctionType)

    nc = bacc.Bacc("TRN2", target_bir_lowering=False, debug=False)
    qt_d = nc.dram_tensor("qt", [BPC, D, LQ], f16, kind="ExternalInput").ap()
    kt_d = nc.dram_tensor("kt", [BPC, D, LK], f16, kind="ExternalInput").ap()
    v_d = nc.dram_tensor("v", [BPC, LK, DV], f32, kind="ExternalInput").ap()
    mt_d = nc.dram_tensor(
        "mt", [LK - XJ * P, LQ], f16, kind="ExternalInput"
    ).ap()
    lm_d = nc.dram_tensor("lm", [XJ * P, LQ], f16, kind="ExternalInput").ap()
    o_d = nc.dram_tensor("o", [BPC, LQ, DV], f16, kind="ExternalOutput").ap()
    aps = (qt_d, kt_d, v_d, mt_d, lm_d, o_d)

    with tile.TileContext(nc) as tc:
        with ExitStack() as ctx:
            from concourse.masks import make_identity

            big = ctx.enter_context(tc.tile_pool(name="big", bufs=1))
            psum_s = ctx.enter_context(
                tc.tile_pool(name="psum_s", bufs=1, space="PSUM")
            )
            psum_o = ctx.enter_context(
                tc.tile_pool(name="psum_o", bufs=1, space="PSUM")
            )
            work = ctx.enter_context(tc.tile_pool(name="work", bufs=8))
            small = ctx.enter_context(tc.tile_pool(name="small", bufs=8))
            consts = ctx.enter_context(tc.tile_pool(name="consts", bufs=1))
            ident = consts.tile([P, P], f32)
            make_identity(nc, ident)
            ident16 = consts.tile([P, P], f16)
            make_identity(nc, ident16)
            pools = (big, psum_s, psum_o, work, small, ident, ident16)
            for _ in range(MAIN_REPS):
                _emit_pass(nc, tc, pools, aps, dts)

    nc.compile()
    return nc


def _get_module():
    global _CACHED
    if _CACHED is None:
        _CACHED = _build_module()
    return _CACHED


def kernel(query, key, value, mask, _trace=False):
    from concourse.bass_utils import run_bass_kernel_spmd

    nc = _get_module()
    in_maps = prep_core_inputs(query, key, value, mask)
    res = run_bass_kernel_spmd(
        nc, in_maps, core_ids=list(range(NCORES)), trace=_trace
    )
    out = np.concatenate(
        [res.results[c]["o"] for c in range(NCORES)], axis=0
    ).astype(np.float32)  # device returns f16; upcast to the contract dtype
    if _trace:
        return out, res
    return out


# revision 87
# speedup vs baseline: 1.8172x; 1.8172x over previous
"""Trainium2 Bass kernel for nn_DotProductAttention_76338748719461.

Attention with a multiplicative mask and softmax over the QUERY axis
(axis=1 of [B, Lq, Lk] scores):

    S[b,q,k]  = (Q[b,q,:] . K[b,k,:]) / 8 + max(log(mask[0,q,k]), F32_MIN)
    A         = softmax(S, axis=q)
    out[b,q,v]= sum_k A[b,q,k] * V[b,k,v]

Key identity: exp(S + log m) = exp(S) * m, so the mask is applied as a
multiply after exp — no log, no additive bias, and mask==0 handled exactly.

Design (per NeuronCore; batch data-parallel over 8 cores, 2 per core):
  * ALL layout work happens on host: Q^T (pre-scaled by 1/8) and K^T in
    f16, V in f32, and the mask TRANSPOSED to [k, q] in f16; output
    returns f16 and is upcast on host. The device does zero transposes
    and zero dtype-conversion DMAs (v1 burned ~33 MB/core of HBM
    traffic and a whole prep phase on mask cast+transpose).
  * Work in the transposed score orientation S_T[k, q], so the softmax
    reduction (over q) is a free-axis reduction.
  * Software pipeline at q-half granularity over 32 (batch, k-tile)
    units: three rotating [128, 1024] PSUM score tiles (6 banks) decouple
    the per-half chain QK (PE, f16) -> exp (ACT, the pacer at ~55us/core
    busy) -> PM = E*mask & row-sum D -> Vp = V/D (Pool normalize_recip)
    -> AV.
  * MIXED mask application (XJ=6): k-tiles j<6 accumulate log(mask) into
    the score PSUM via a PE identity-matmul (exp output IS the masked
    weight; one DVE TS-accum for D); tiles j>=6 use the DVE full-width
    multiply + TS-accum. This balances DVE (~40us) against spare PE
    (~55us) — DVE f16 ops run at 1-2x on real HW, not the model's 2-4x,
    and an all-DVE mask path co-paces with ACT at ~100us/pass.
  * AV with SWAPPED operand roles: stationary = PM chunk [128k, 128q],
    moving = Vp [128k, 64v] => out[q, v] accumulates directly in PSUM
    ([128, 16, 64] f32 = 2 banks, lazy-zero bank groups); no output
    transposes. AV for unit u is emitted at unit u+5 so the in-order PE
    stream never stalls on the denominator chain.
  * ~20 dummy PE transposes at kernel start ride the input-DMA wait to
    finish the 3us p-state ramp before the first real matmul.

Measured (8 cores, in-NEFF repetition differential, paired median):
~81us steady-state per pass (v1 baseline: 139us differential, 207us
harness); scale-relative absmax error 7.9e-4 vs the fp32 reference.
Ablations show the DVE is the HW pacer (~55us busy; exp on ACT hides
completely underneath), so denominators use READ-ONLY tensor_reduce
(not TS-accum, which re-writes the tensor) and the PSUM drain runs on
ACT via activation(Copy).

Hard-won HW facts (cost model/CoreSim do NOT flag these):
  * InstTensorTensorReduce (ISA op) crashes the DVE exec unit on TRN2.
  * activation(accum_out=...) is correct on HW ONLY if the accumulator
    is read >=1 unit later (the write lands after the completion sem;
    an immediate read races -> garbage), and it serializes the ACT
    pipeline (~2x per-pass cost when used on 24 exp instrs — measured
    162us vs 81us). Not worth it; reduce on DVE instead.
  * TensorScalar/free-axis TensorReduce are illegal opcodes on Pool;
    Pool is the GPSIMD DSP engine (custom ucode ops only, no PSUM).
  * DMAs issued from the ACT queue stall the exp stream badly.
  * gpsimd-queue DMAs burn ~1.3us of Pool ENGINE time each (SWDGE).
"""

import os
import numpy as np

B, LQ, LK, D, DV = 16, 2048, 2048, 64, 64
NCORES = 8
BPC = B // NCORES  # batches per core
P = 128
CH = 512  # QK matmul moving chunk (one PSUM bank of fp32)
NT_Q = LQ // P  # 16
NT_K = LK // P  # 16
SCALE = 1.0 / 8.0  # 1/sqrt(64), folded into host-side Q^T prep

MAIN_REPS = int(os.environ.get("MAIN_REPS", "1"))  # repeat body (timing builds)
ABLATE = os.environ.get("ABLATE", "")  # timing-only ablations: nodve|noact|noav
# k-tiles j < XJ use the "additive" mask method: log(mask) is accumulated
# into the score PSUM by a PE identity-matmul and ACT's exp accumulates the
# softmax denominator itself — zero DVE work for those units. The rest use
# the DVE multiply path. This balances DVE (whose f16 ops run at 1-2x on
# real HW, not the cost model's 2-4x) against the PE's spare throughput.
XJ = int(os.environ.get("XJ", "6"))

_CACHED = None


def prep_core_inputs(query, key, value, mask):
    """Host-side layout prep: per-core input dicts for the device binary.

    qt: [BPC, 64, 2048] f16 = (Q/8)^T     kt: [BPC, 64, 2048] f16 = K^T
    v:  [BPC, 2048, 64] f16               mt: [2048, 2048] f16 = mask[0]^T
    """
    query = np.asarray(query, dtype=np.float32)
    key = np.asarray(key, dtype=np.float32)
    value = np.asarray(value, dtype=np.float32)
    mask = np.asarray(mask, dtype=np.float32)

    q16 = (query.transpose(0, 2, 1) * np.float32(SCALE)).astype(np.float16)
    k16 = key.transpose(0, 2, 1).astype(np.float16)
    mt = mask[0].T  # [k, q]
    m16 = mt[XJ * P :].astype(np.float16)  # multiply-path tiles (j >= XJ)
    with np.errstate(divide="ignore"):
        lm16 = np.log(mt[: XJ * P]).astype(np.float16)  # additive-path tiles
    return [
        {
            "qt": np.ascontiguousarray(q16[c * BPC : (c + 1) * BPC]),
            "kt": np.ascontiguousarray(k16[c * BPC : (c + 1) * BPC]),
            "v": np.ascontiguousarray(value[c * BPC : (c + 1) * BPC]),
            "mt": m16,
            "lm": lm16,
        }
        for c in range(NCORES)
    ]


def _emit_av(nc, O, PM, Vp, j):
    """AV with swapped roles: stationary PM chunk [128k, 128q], moving
    Vp [128k, 64v] -> out[q, v] accumulates directly in PSUM (2 banks).

    PSUM accumulation groups are bank-granular (2 KB zero regions, 8
    chunks of [128, 64] f32 per bank): open each bank's group with
    start=True on its first chunk at j==0 (lazy-zeroes the whole bank;
    later j==0 chunks land on pending-zero bytes and replace), close it
    with stop=True on its last chunk at j==15."""
    from concourse.bass import ds

    for t in range(NT_Q):
        nc.tensor.matmul(
            O[:, t, :],
            PM[:, ds(P * t, P)],
            Vp[:],
            start=(j == 0 and t % 8 == 0),
            stop=(j == NT_K - 1 and t % 8 == 7),
            skip_group_check=True,
        )


def _emit_out(nc, work, o_d, pO, pb, ident, psum_o):
    """Evacuate the [q, v]-oriented PSUM accumulator: DVE copy to SBUF
    (f32 PSUM -> f16, host upcasts), then DMA on the sync queue — in two
    halves so the copy and the DMA pipeline."""
    import concourse.mybir as mybir
    from concourse.bass import ds

    import concourse.mybir as _mb

    out_sb = work.tile(
        [P, NT_Q, DV], mybir.dt.float16, tag="osb", bufs=2, name="out_sb"
    )
    dst = o_d[pb].rearrange("(t p) d -> p t d", p=P)
    half = NT_Q // 2
    for g in range(2):
        gs = ds(g * half, half)
        # evacuate on ACT (it has slack; DVE is the pacer)
        nc.scalar.activation(
            out_sb[:, gs, :], pO[:, gs, :],
            _mb.ActivationFunctionType.Copy,
        )
        nc.sync.dma_start(dst[:, gs, :], out_sb[:, gs, :])


def _emit_pass(nc, tc, pools, aps, dts):
    """One full pass: input DMAs + 2 batches x 16 k-tiles + output DMAs."""
    import concourse.mybir as mybir
    from concourse.bass import ds, ts

    qt_d, kt_d, v_d, mt_d, lm_d, o_d = aps
    f32, f16, AF = dts
    ALU = mybir.AluOpType
    big, psum_s, psum_o, work, small, ident, ident16 = pools
    HF0 = LQ // 2

    mT = big.tile([P, NT_K - XJ, LQ], f16, tag="mT", name="mT")
    lmT = big.tile([P, XJ, LQ], f16, tag="lmT", name="lmT")
    QT = big.tile([D, BPC, LQ], f16, tag="QT", name="QT")
    KT = big.tile([D, BPC, LK], f16, tag="KT", name="KT")
    Vn = big.tile([P, BPC, NT_K, DV], f32, tag="Vn", name="Vn")

    # ALL input DMAs go on the sync queue (hardware DGE — the gpsimd
    # queue's software DGE burns ~1.3us of Pool ENGINE time per
    # transfer), hand-ordered so each tile lands just before its first
    # use: batch 0's K/Q first (first QK), then early mask tiles (tile j
    # is consumed at ~2.2us*j), V0 (first Vp), batch 1's K/Q, and the
    # remaining mask tiles, which stay ahead of consumption from there.
    def dma_v(b):
        nc.sync.dma_start(
            Vn[:, b, :, :], v_d[b].rearrange("(t p) d -> p t d", p=P)
        )

    def dma_m(j):
        # all mask tiles on the SP hardware-DGE queue (DMAs issued from
        # the ACT queue measurably stall the exp stream — do not split)
        if j < XJ:
            nc.sync.dma_start(lmT[:, j, :], lm_d[ds(P * j, P), :])
        else:
            nc.sync.dma_start(
                mT[:, j - XJ, :], mt_d[ds(P * (j - XJ), P), :]
            )

    # just the slices the first QK touches (~150 KB), so the first score
    # matmul can issue ~1us earlier than a full-tile load would allow
    nc.sync.dma_start(KT[:, 0, ds(0, P)], kt_d[0][:, ds(0, P)])
    nc.sync.dma_start(QT[:, 0, ds(0, HF0)], qt_d[0][:, ds(0, HF0)])
    nc.sync.dma_start(KT[:, 0, ds(P, LK - P)], kt_d[0][:, ds(P, LK - P)])
    nc.sync.dma_start(QT[:, 0, ds(HF0, HF0)], qt_d[0][:, ds(HF0, HF0)])
    dma_m(0)
    dma_m(1)
    dma_m(2)
    dma_v(0)
    nc.sync.dma_start(KT[:, 1, :], kt_d[1])
    nc.sync.dma_start(QT[:, 1, :], qt_d[1])
    dma_m(3)
    dma_v(1)
    for j in range(4, NT_K):
        dma_m(j)

    # Software pipeline at q-half granularity. The two q-halves of the
    # score tile live in SEPARATE PSUM tiles (2 banks each) so dependency
    # tracking is per-half: QK for half t+1 is emitted one ACT-slot ahead
    # of ACT for half t, so the exp stream never waits on the PE.
    # Cross-engine consumers are emitted with a lag so no in-order engine
    # stream ever blocks on a slow producer:
    #   - reciprocal/Vp for unit u are emitted during unit u+1,
    #   - AV matmuls for unit u are emitted during unit u+DEFER (the
    #     denominator chain ACT->DVE->Pool->recip->Vp is ~2.5 units long
    #     and the in-order PE stream would stall on the AV Ldweights).
    # Three rotating half-score tiles (2 banks each; the swapped AV's
    # 2-bank O frees the room): QK for half t+1 only has a WAR against
    # ACT of half t-2, giving the PE a full extra ACT slot of slack.
    HF = LQ // 2
    NS = 3
    S3 = [
        psum_s.tile([P, HF], f32, tag=f"s{h}", name=f"S{h}")
        for h in range(NS)
    ]
    h0s, h1s = ds(0, HF), ds(HF, HF)
    DEFER = 5

    # PE p-state warmup: ~20 dep-free dummy transposes into S3[0] (junk;
    # overwritten by the first QK) run during the input-DMA wait so the
    # 3us ramp to full clock is over by the time real matmuls issue.
    for _ in range(20):
        nc.tensor.transpose(
            S3[0][0:DV, 0:DV], ident[0:DV, 0:DV], ident[0:DV, 0:DV]
        )

    units = [(b, j) for b in range(BPC) for j in range(NT_K)]
    NU = len(units)
    ctx = {}  # u -> dict of tiles
    O_of = {}  # b -> O psum tile

    def emit_qk(t):
        u, h = t // 2, t % 2
        b, j = units[u]
        addm = j < XJ  # additive-mask unit: accumulate log(mask) on PE
        for c in range(2):
            nc.tensor.matmul(
                S3[t % NS][:, ts(c, CH)],
                KT[:, b, ds(P * j, P)],
                QT[:, b, ds(HF * h + CH * c, CH)],
                start=True,
                stop=not addm,
            )
        if addm:
            for c in range(2):
                nc.tensor.matmul(
                    S3[t % NS][:, ts(c, CH)],
                    ident16[:],
                    lmT[:, j, ds(HF * h + CH * c, CH)],
                    start=False,
                    stop=True,
                )

    emit_qk(0)
    for t in range(2 * (NU + DEFER) + 2):
        u, h = t // 2, t % 2
        if t + 1 < 2 * NU:
            emit_qk(t + 1)
        if h == 1 and u - DEFER >= 0 and u - DEFER < NU:
            ua = u - DEFER
            ca = ctx[ua]
            ba, ja = units[ua]
            if ABLATE != "noav":
                _emit_av(nc, O_of[ba], ca["PM"], ca["Vp"], ja)
            if ja == NT_K - 1:
                _emit_out(nc, work, o_d, O_of[ba], ba, ident, psum_o)
            del ctx[ua]
        if u >= NU:
            continue
        b, j = units[u]
        addm = j < XJ
        if h == 0:
            E = work.tile([P, LQ], f16, tag="e", name="E")
            ctx[u] = {"E": E}
            if not addm:
                ctx[u]["PM"] = work.tile([P, LQ], f16, tag="pm", name="PM")
            if b not in O_of:
                O_of[b] = psum_o.tile(
                    [P, NT_Q, DV], f32, tag="o", name=f"O{b}"
                )
                if ABLATE == "noav":
                    nc.vector.memset(O_of[b][:, 0, :], 0.0)
            if ABLATE != "noact":
                nc.scalar.activation(E[:, h0s], S3[t % NS][:], AF.Exp)
            else:
                nc.vector.memset(E[:, ds(0, 32)], 1.0)
        else:
            E = ctx[u]["E"]
            if ABLATE != "noact":
                nc.scalar.activation(E[:, h1s], S3[t % NS][:], AF.Exp)
            # lagged Vp for the PREVIOUS unit: one Pool-local
            # normalize_recip (Vp = V / D), so the denominator tail never
            # leaves the Pool engine's in-order stream.
            if u - 1 >= 0 and "Dsum" in ctx.get(u - 1, {}):
                cp = ctx[u - 1]
                bp, jp = units[u - 1]
                Vp = small.tile([P, DV], f16, tag="vp", name="Vp")
                nc.gpsimd.normalize_recip(
                    Vp[:], Vn[:, bp, jp, :], cp["Dsum"][:]
                )
                cp["Vp"] = Vp
            Dsum = small.tile([P, 1], f32, tag="d", name="Dsum")
            if addm:
                # additive-mask unit: the masked weights ARE exp's output;
                # one READ-ONLY reduction computes the denominator (a
                # TS-accum would re-write the full tensor — two SBUF
                # streams instead of one).
                nc.vector.tensor_reduce(
                    Dsum[:], E[:], axis=mybir.AxisListType.X, op=ALU.add
                )
                ctx[u]["PM"] = E
            elif ABLATE != "nodve":
                from concourse.dve_ops import TENSOR_TENSOR_REDUCE

                PM = ctx[u]["PM"]
                # one fused custom-DVE op: PM = E*mask AND D = row-sum —
                # halves the instruction/semaphore count on the pacer
                # (the CUSTOM ucode op; the same-named ISA op crashes)
                nc.vector._custom_dve(
                    TENSOR_TENSOR_REDUCE,
                    out=PM[:], in0=E[:], in1=mT[:, j - XJ, :],
                    s0=0.0, s1=1.0, accum_out=Dsum[:],
                )
            else:  # timing ablation: tiny writes so tiles count as allocated
                nc.vector.memset(ctx[u]["PM"][:, ds(0, 32)], 1.0)
                nc.vector.memset(Dsum[:], 1.0)
            ctx[u]["Dsum"] = Dsum
            if u == NU - 1:  # no u+1 step will emit our Vp
                Vp = small.tile([P, DV], f16, tag="vp", name="Vp")
                nc.gpsimd.normalize_recip(Vp[:], Vn[:, b, j, :], Dsum[:])
                ctx[u]["Vp"] = Vp


def _build_module():
    import concourse.mybir as mybir
    import concourse.tile as tile
    from concourse import bacc
    from contextlib import ExitStack

    f32 = mybir.dt.float32
    f16 = mybir.dt.float16
    dts = (f32, f16, mybir.ActivationFunctionType)

    nc = bacc.Bacc("TRN2", target_bir_lowering=False, debug=False)
    qt_d = nc.dram_tensor("qt", [BPC, D, LQ], f16, kind="ExternalInput").ap()
    kt_d = nc.dram_tensor("kt", [BPC, D, LK], f16, kind="ExternalInput").ap()
    v_d = nc.dram_tensor("v", [BPC, LK, DV], f32, kind="ExternalInput").ap()
    mt_d = nc.dram_tensor(
        "mt", [LK - XJ * P, LQ], f16, kind="ExternalInput"
    ).ap()
    lm_d = nc.dram_tensor("lm", [XJ * P, LQ], f16, kind="ExternalInput").ap()
    o_d = nc.dram_tensor("o", [BPC, LQ, DV], f16, kind="ExternalOutput").ap()
    aps = (qt_d, kt_d, v_d, mt_d, lm_d, o_d)

    with tile.TileContext(nc) as tc:
        with ExitStack() as ctx:
            from concourse.masks import make_identity

            big = ctx.enter_context(tc.tile_pool(name="big", bufs=1))
            psum_s = ctx.enter_context(
                tc.tile_pool(name="psum_s", bufs=1, space="PSUM")
            )
            psum_o = ctx.enter_context(
                tc.tile_pool(name="psum_o", bufs=1, space="PSUM")
            )
            work = ctx.enter_context(tc.tile_pool(name="work", bufs=8))
            small = ctx.enter_context(tc.tile_pool(name="small", bufs=8))
            consts = ctx.enter_context(tc.tile_pool(name="consts", bufs=1))
            ident = consts.tile([P, P], f32)
            make_identity(nc, ident)
            ident16 = consts.tile([P, P], f16)
            make_identity(nc, ident16)
            pools = (big, psum_s, psum_o, work, small, ident, ident16)
            for _ in range(MAIN_REPS):
                _emit_pass(nc, tc, pools, aps, dts)

    nc.compile()
    return nc


def _get_module():
    global _CACHED
    if _CACHED is None:
        _CACHED = _build_module()
    return _CACHED


def kernel(query, key, value, mask, _trace=False):
    from concourse.bass_utils import run_bass_kernel_spmd

    nc = _get_module()
    in_maps = prep_core_inputs(query, key, value, mask)
    res = run_bass_kernel_spmd(
        nc, in_maps, core_ids=list(range(NCORES)), trace=_trace
    )
    out = np.concatenate(
        [res.results[c]["o"] for c in range(NCORES)], axis=0
    ).astype(np.float32)  # device returns f16; upcast to the contract dtype
    if _trace:
        return out, res
    return out


# revision 90
# speedup vs baseline: 1.8637x; 1.0256x over previous
"""Trainium2 Bass kernel for nn_DotProductAttention_76338748719461.

Attention with a multiplicative mask and softmax over the QUERY axis
(axis=1 of [B, Lq, Lk] scores):

    S[b,q,k]  = (Q[b,q,:] . K[b,k,:]) / 8 + max(log(mask[0,q,k]), F32_MIN)
    A         = softmax(S, axis=q)
    out[b,q,v]= sum_k A[b,q,k] * V[b,k,v]

Key identity: exp(S + log m) = exp(S) * m, so the mask is applied as a
multiply after exp — no log, no additive bias, and mask==0 handled exactly.

Design (per NeuronCore; batch data-parallel over 8 cores, 2 per core):
  * ALL layout work happens on host: Q^T (pre-scaled by 1/8) and K^T in
    f16, V in f32, and the mask TRANSPOSED to [k, q] in f16; output
    returns f16 and is upcast on host. The device does zero transposes
    and zero dtype-conversion DMAs (v1 burned ~33 MB/core of HBM
    traffic and a whole prep phase on mask cast+transpose).
  * Work in the transposed score orientation S_T[k, q], so the softmax
    reduction (over q) is a free-axis reduction.
  * Software pipeline at q-half granularity over 32 (batch, k-tile)
    units: three rotating [128, 1024] PSUM score tiles (6 banks) decouple
    the per-half chain QK (PE, f16) -> exp (ACT, the pacer at ~55us/core
    busy) -> PM = E*mask & row-sum D -> Vp = V/D (Pool normalize_recip)
    -> AV.
  * MIXED mask application (XJ=6): k-tiles j<6 accumulate log(mask) into
    the score PSUM via a PE identity-matmul (exp output IS the masked
    weight; one DVE TS-accum for D); tiles j>=6 use the DVE full-width
    multiply + TS-accum. This balances DVE (~40us) against spare PE
    (~55us) — DVE f16 ops run at 1-2x on real HW, not the model's 2-4x,
    and an all-DVE mask path co-paces with ACT at ~100us/pass.
  * AV with SWAPPED operand roles: stationary = PM chunk [128k, 128q],
    moving = Vp [128k, 64v] => out[q, v] accumulates directly in PSUM
    ([128, 16, 64] f32 = 2 banks, lazy-zero bank groups); no output
    transposes. AV for unit u is emitted at unit u+5 so the in-order PE
    stream never stalls on the denominator chain.
  * ~20 dummy PE transposes at kernel start ride the input-DMA wait to
    finish the 3us p-state ramp before the first real matmul.

Measured (8 cores, in-NEFF repetition differential, paired median):
~81us steady-state per pass (v1 baseline: 139us differential, 207us
harness); scale-relative absmax error 7.9e-4 vs the fp32 reference.
Ablations show the DVE is the HW pacer (~55us busy; exp on ACT hides
completely underneath), so denominators use READ-ONLY tensor_reduce
(not TS-accum, which re-writes the tensor) and the PSUM drain runs on
ACT via activation(Copy).

Hard-won HW facts (cost model/CoreSim do NOT flag these):
  * InstTensorTensorReduce (ISA op) crashes the DVE exec unit on TRN2.
  * activation(accum_out=...) is correct on HW ONLY if the accumulator
    is read >=1 unit later (the write lands after the completion sem;
    an immediate read races -> garbage), and it serializes the ACT
    pipeline (~2x per-pass cost when used on 24 exp instrs — measured
    162us vs 81us). Not worth it; reduce on DVE instead.
  * TensorScalar/free-axis TensorReduce are illegal opcodes on Pool;
    Pool is the GPSIMD DSP engine (custom ucode ops only, no PSUM).
  * DMAs issued from the ACT queue stall the exp stream badly.
  * gpsimd-queue DMAs burn ~1.3us of Pool ENGINE time each (SWDGE).
  * Inputs generated on the neuron backend (vs CPU) can contain EXACT
    mask zeros; log(0)=-inf through the identity-matmul add makes
    0*inf=NaN — the host prep clamps log(mask) to -60000 (exp still
    underflows to exactly 0). Always validate with device-generated
    inputs, not just CPU-generated ones.
"""

import os
import numpy as np

B, LQ, LK, D, DV = 16, 2048, 2048, 64, 64
NCORES = 8
BPC = B // NCORES  # batches per core
P = 128
CH = 512  # QK matmul moving chunk (one PSUM bank of fp32)
NT_Q = LQ // P  # 16
NT_K = LK // P  # 16
SCALE = 1.0 / 8.0  # 1/sqrt(64), folded into host-side Q^T prep

MAIN_REPS = int(os.environ.get("MAIN_REPS", "1"))  # repeat body (timing builds)
ABLATE = os.environ.get("ABLATE", "")  # timing-only ablations: nodve|noact|noav
# k-tiles j < XJ use the "additive" mask method: log(mask) is accumulated
# into the score PSUM by a PE identity-matmul and ACT's exp accumulates the
# softmax denominator itself — zero DVE work for those units. The rest use
# the DVE multiply path. This balances DVE (whose f16 ops run at 1-2x on
# real HW, not the cost model's 2-4x) against the PE's spare throughput.
XJ = int(os.environ.get("XJ", "6"))

_CACHED = None


def prep_core_inputs(query, key, value, mask):
    """Host-side layout prep: per-core input dicts for the device binary.

    qt: [BPC, 64, 2048] f16 = (Q/8)^T     kt: [BPC, 64, 2048] f16 = K^T
    v:  [BPC, 2048, 64] f16               mt: [2048, 2048] f16 = mask[0]^T
    """
    query = np.asarray(query, dtype=np.float32)
    key = np.asarray(key, dtype=np.float32)
    value = np.asarray(value, dtype=np.float32)
    mask = np.asarray(mask, dtype=np.float32)

    q16 = (query.transpose(0, 2, 1) * np.float32(SCALE)).astype(np.float16)
    k16 = key.transpose(0, 2, 1).astype(np.float16)
    mt = mask[0].T  # [k, q]
    m16 = mt[XJ * P :].astype(np.float16)  # multiply-path tiles (j >= XJ)
    with np.errstate(divide="ignore"):
        # additive-path tiles; clamp FINITE: mask==0 -> log = -inf would
        # hit 0*inf=NaN inside the identity-matmul add. exp(S-60000)
        # underflows to exactly 0, matching the reference's clamp.
        lm16 = np.maximum(np.log(mt[: XJ * P]), np.float32(-60000.0)).astype(
            np.float16
        )
    return [
        {
            "qt": np.ascontiguousarray(q16[c * BPC : (c + 1) * BPC]),
            "kt": np.ascontiguousarray(k16[c * BPC : (c + 1) * BPC]),
            "v": np.ascontiguousarray(value[c * BPC : (c + 1) * BPC]),
            "mt": m16,
            "lm": lm16,
        }
        for c in range(NCORES)
    ]


def _emit_av(nc, O, PM, Vp, j):
    """AV with swapped roles: stationary PM chunk [128k, 128q], moving
    Vp [128k, 64v] -> out[q, v] accumulates directly in PSUM (2 banks).

    PSUM accumulation groups are bank-granular (2 KB zero regions, 8
    chunks of [128, 64] f32 per bank): open each bank's group with
    start=True on its first chunk at j==0 (lazy-zeroes the whole bank;
    later j==0 chunks land on pending-zero bytes and replace), close it
    with stop=True on its last chunk at j==15."""
    from concourse.bass import ds

    for t in range(NT_Q):
        nc.tensor.matmul(
            O[:, t, :],
            PM[:, ds(P * t, P)],
            Vp[:],
            start=(j == 0 and t % 8 == 0),
            stop=(j == NT_K - 1 and t % 8 == 7),
            skip_group_check=True,
        )


def _emit_out(nc, work, o_d, pO, pb, ident, psum_o):
    """Evacuate the [q, v]-oriented PSUM accumulator: DVE copy to SBUF
    (f32 PSUM -> f16, host upcasts), then DMA on the sync queue — in two
    halves so the copy and the DMA pipeline."""
    import concourse.mybir as mybir
    from concourse.bass import ds

    import concourse.mybir as _mb

    out_sb = work.tile(
        [P, NT_Q, DV], mybir.dt.float16, tag="osb", bufs=2, name="out_sb"
    )
    dst = o_d[pb].rearrange("(t p) d -> p t d", p=P)
    half = NT_Q // 2
    for g in range(2):
        gs = ds(g * half, half)
        # evacuate on ACT (it has slack; DVE is the pacer)
        nc.scalar.activation(
            out_sb[:, gs, :], pO[:, gs, :],
            _mb.ActivationFunctionType.Copy,
        )
        nc.sync.dma_start(dst[:, gs, :], out_sb[:, gs, :])


def _emit_pass(nc, tc, pools, aps, dts):
    """One full pass: input DMAs + 2 batches x 16 k-tiles + output DMAs."""
    import concourse.mybir as mybir
    from concourse.bass import ds, ts

    qt_d, kt_d, v_d, mt_d, lm_d, o_d = aps
    f32, f16, AF = dts
    ALU = mybir.AluOpType
    big, psum_s, psum_o, work, small, ident, ident16 = pools
    HF0 = LQ // 2

    mT = big.tile([P, NT_K - XJ, LQ], f16, tag="mT", name="mT")
    lmT = big.tile([P, XJ, LQ], f16, tag="lmT", name="lmT")
    QT = big.tile([D, BPC, LQ], f16, tag="QT", name="QT")
    KT = big.tile([D, BPC, LK], f16, tag="KT", name="KT")
    Vn = big.tile([P, BPC, NT_K, DV], f32, tag="Vn", name="Vn")

    # ALL input DMAs go on the sync queue (hardware DGE — the gpsimd
    # queue's software DGE burns ~1.3us of Pool ENGINE time per
    # transfer), hand-ordered so each tile lands just before its first
    # use: batch 0's K/Q first (first QK), then early mask tiles (tile j
    # is consumed at ~2.2us*j), V0 (first Vp), batch 1's K/Q, and the
    # remaining mask tiles, which stay ahead of consumption from there.
    def dma_v(b):
        nc.sync.dma_start(
            Vn[:, b, :, :], v_d[b].rearrange("(t p) d -> p t d", p=P)
        )

    def dma_m(j):
        # all mask tiles on the SP hardware-DGE queue (DMAs issued from
        # the ACT queue measurably stall the exp stream — do not split)
        if j < XJ:
            nc.sync.dma_start(lmT[:, j, :], lm_d[ds(P * j, P), :])
        else:
            nc.sync.dma_start(
                mT[:, j - XJ, :], mt_d[ds(P * (j - XJ), P), :]
            )

    # just the slices the first QK touches (~150 KB), so the first score
    # matmul can issue ~1us earlier than a full-tile load would allow
    nc.sync.dma_start(KT[:, 0, ds(0, P)], kt_d[0][:, ds(0, P)])
    nc.sync.dma_start(QT[:, 0, ds(0, HF0)], qt_d[0][:, ds(0, HF0)])
    nc.sync.dma_start(KT[:, 0, ds(P, LK - P)], kt_d[0][:, ds(P, LK - P)])
    nc.sync.dma_start(QT[:, 0, ds(HF0, HF0)], qt_d[0][:, ds(HF0, HF0)])
    dma_m(0)
    dma_m(1)
    dma_m(2)
    dma_v(0)
    nc.sync.dma_start(KT[:, 1, :], kt_d[1])
    nc.sync.dma_start(QT[:, 1, :], qt_d[1])
    dma_m(3)
    dma_v(1)
    for j in range(4, NT_K):
        dma_m(j)

    # Software pipeline at q-half granularity. The two q-halves of the
    # score tile live in SEPARATE PSUM tiles (2 banks each) so dependency
    # tracking is per-half: QK for half t+1 is emitted one ACT-slot ahead
    # of ACT for half t, so the exp stream never waits on the PE.
    # Cross-engine consumers are emitted with a lag so no in-order engine
    # stream ever blocks on a slow producer:
    #   - reciprocal/Vp for unit u are emitted during unit u+1,
    #   - AV matmuls for unit u are emitted during unit u+DEFER (the
    #     denominator chain ACT->DVE->Pool->recip->Vp is ~2.5 units long
    #     and the in-order PE stream would stall on the AV Ldweights).
    # Three rotating half-score tiles (2 banks each; the swapped AV's
    # 2-bank O frees the room): QK for half t+1 only has a WAR against
    # ACT of half t-2, giving the PE a full extra ACT slot of slack.
    HF = LQ // 2
    NS = 3
    S3 = [
        psum_s.tile([P, HF], f32, tag=f"s{h}", name=f"S{h}")
        for h in range(NS)
    ]
    h0s, h1s = ds(0, HF), ds(HF, HF)
    DEFER = 5

    # PE p-state warmup: ~20 dep-free dummy transposes into S3[0] (junk;
    # overwritten by the first QK) run during the input-DMA wait so the
    # 3us ramp to full clock is over by the time real matmuls issue.
    for _ in range(20):
        nc.tensor.transpose(
            S3[0][0:DV, 0:DV], ident[0:DV, 0:DV], ident[0:DV, 0:DV]
        )

    units = [(b, j) for b in range(BPC) for j in range(NT_K)]
    NU = len(units)
    ctx = {}  # u -> dict of tiles
    O_of = {}  # b -> O psum tile

    def emit_qk(t):
        u, h = t // 2, t % 2
        b, j = units[u]
        addm = j < XJ  # additive-mask unit: accumulate log(mask) on PE
        for c in range(2):
            nc.tensor.matmul(
                S3[t % NS][:, ts(c, CH)],
                KT[:, b, ds(P * j, P)],
                QT[:, b, ds(HF * h + CH * c, CH)],
                start=True,
                stop=not addm,
            )
        if addm:
            for c in range(2):
                nc.tensor.matmul(
                    S3[t % NS][:, ts(c, CH)],
                    ident16[:],
                    lmT[:, j, ds(HF * h + CH * c, CH)],
                    start=False,
                    stop=True,
                )

    emit_qk(0)
    for t in range(2 * (NU + DEFER) + 2):
        u, h = t // 2, t % 2
        if t + 1 < 2 * NU:
            emit_qk(t + 1)
        if h == 1 and u - DEFER >= 0 and u - DEFER < NU:
            ua = u - DEFER
            ca = ctx[ua]
            ba, ja = units[ua]
            if ABLATE != "noav":
                _emit_av(nc, O_of[ba], ca["PM"], ca["Vp"], ja)
            if ja == NT_K - 1:
                _emit_out(nc, work, o_d, O_of[ba], ba, ident, psum_o)
            del ctx[ua]
        if u >= NU:
            continue
        b, j = units[u]
        addm = j < XJ
        if h == 0:
            E = work.tile([P, LQ], f16, tag="e", name="E")
            ctx[u] = {"E": E}
            if not addm:
                ctx[u]["PM"] = work.tile([P, LQ], f16, tag="pm", name="PM")
            if b not in O_of:
                O_of[b] = psum_o.tile(
                    [P, NT_Q, DV], f32, tag="o", name=f"O{b}"
                )
                if ABLATE == "noav":
                    nc.vector.memset(O_of[b][:, 0, :], 0.0)
            if ABLATE != "noact":
                nc.scalar.activation(E[:, h0s], S3[t % NS][:], AF.Exp)
            else:
                nc.vector.memset(E[:, ds(0, 32)], 1.0)
        else:
            E = ctx[u]["E"]
            if ABLATE != "noact":
                nc.scalar.activation(E[:, h1s], S3[t % NS][:], AF.Exp)
            # lagged Vp for the PREVIOUS unit: one Pool-local
            # normalize_recip (Vp = V / D), so the denominator tail never
            # leaves the Pool engine's in-order stream.
            if u - 1 >= 0 and "Dsum" in ctx.get(u - 1, {}):
                cp = ctx[u - 1]
                bp, jp = units[u - 1]
                Vp = small.tile([P, DV], f16, tag="vp", name="Vp")
                nc.gpsimd.normalize_recip(
                    Vp[:], Vn[:, bp, jp, :], cp["Dsum"][:]
                )
                cp["Vp"] = Vp
            Dsum = small.tile([P, 1], f32, tag="d", name="Dsum")
            if addm:
                # additive-mask unit: the masked weights ARE exp's output;
                # one READ-ONLY reduction computes the denominator (a
                # TS-accum would re-write the full tensor — two SBUF
                # streams instead of one).
                nc.vector.tensor_reduce(
                    Dsum[:], E[:], axis=mybir.AxisListType.X, op=ALU.add
                )
                ctx[u]["PM"] = E
            elif ABLATE != "nodve":
                PM = ctx[u]["PM"]
                # DVE: full-width mask multiply, then read-only reduction
                nc.vector.tensor_tensor(
                    PM[:], E[:], mT[:, j - XJ, :], ALU.mult
                )
                nc.vector.tensor_reduce(
                    Dsum[:], PM[:], axis=mybir.AxisListType.X, op=ALU.add
                )
            else:  # timing ablation: tiny writes so tiles count as allocated
                nc.vector.memset(ctx[u]["PM"][:, ds(0, 32)], 1.0)
                nc.vector.memset(Dsum[:], 1.0)
            ctx[u]["Dsum"] = Dsum
            if u == NU - 1:  # no u+1 step will emit our Vp
                Vp = small.tile([P, DV], f16, tag="vp", name="Vp")
                nc.gpsimd.normalize_recip(Vp[:], Vn[:, b, j, :], Dsum[:])
                ctx[u]["Vp"] = Vp


def _build_module():
    import concourse.mybir as mybir
    import concourse.tile as tile
    from concourse import bacc
    from contextlib import ExitStack

    f32 = mybir.dt.float32
    f16 = mybir.dt.float16
    dts = (f32, f16, mybir.ActivationFunctionType)

    nc = bacc.Bacc("TRN2", target_bir_lowering=False, debug=False)
    qt_d = nc.dram_tensor("qt", [BPC, D, LQ], f16, kind="ExternalInput").ap()
    kt_d = nc.dram_tensor("kt", [BPC, D, LK], f16, kind="ExternalInput").ap()
    v_d = nc.dram_tensor("v", [BPC, LK, DV], f32, kind="ExternalInput").ap()
    mt_d = nc.dram_tensor(
        "mt", [LK - XJ * P, LQ], f16, kind="ExternalInput"
    ).ap()
    lm_d = nc.dram_tensor("lm", [XJ * P, LQ], f16, kind="ExternalInput").ap()
    o_d = nc.dram_tensor("o", [BPC, LQ, DV], f16, kind="ExternalOutput").ap()
    aps = (qt_d, kt_d, v_d, mt_d, lm_d, o_d)

    with tile.TileContext(nc) as tc:
        with ExitStack() as ctx:
            from concourse.masks import make_identity

            big = ctx.enter_context(tc.tile_pool(name="big", bufs=1))
            psum_s = ctx.enter_context(
                tc.tile_pool(name="psum_s", bufs=1, space="PSUM")
            )
            psum_o = ctx.enter_context(
                tc.tile_pool(name="psum_o", bufs=1, space="PSUM")
            )
            work = ctx.enter_context(tc.tile_pool(name="work", bufs=8))
            small = ctx.enter_context(tc.tile_pool(name="small", bufs=8))
            consts = ctx.enter_context(tc.tile_pool(name="consts", bufs=1))
            ident = consts.tile([P, P], f32)
            make_identity(nc, ident)
            ident16 = consts.tile([P, P], f16)
            make_identity(nc, ident16)
            pools = (big, psum_s, psum_o, work, small, ident, ident16)
            for _ in range(MAIN_REPS):
                _emit_pass(nc, tc, pools, aps, dts)

    nc.compile()
    return nc


def _get_module():
    global _CACHED
    if _CACHED is None:
        _CACHED = _build_module()
    return _CACHED


def kernel(query, key, value, mask, _trace=False):
    from concourse.bass_utils import run_bass_kernel_spmd

    nc = _get_module()
    in_maps = prep_core_inputs(query, key, value, mask)
    res = run_bass_kernel_spmd(
        nc, in_maps, core_ids=list(range(NCORES)), trace=_trace
    )
    out = np.concatenate(
        [res.results[c]["o"] for c in range(NCORES)], axis=0
    ).astype(np.float32)  # device returns f16; upcast to the contract dtype
    if _trace:
        return out, res
    return out


# revision 92
# speedup vs baseline: 2.1331x; 1.1445x over previous
"""Trainium2 Bass kernel for nn_DotProductAttention_76338748719461.

Attention with a multiplicative mask and softmax over the QUERY axis
(axis=1 of [B, Lq, Lk] scores):

    S[b,q,k]  = (Q[b,q,:] . K[b,k,:]) / 8 + max(log(mask[0,q,k]), F32_MIN)
    A         = softmax(S, axis=q)
    out[b,q,v]= sum_k A[b,q,k] * V[b,k,v]

Key identity: exp(S + log m) = exp(S) * m, so the mask is applied as a
multiply after exp — no log, no additive bias, and mask==0 handled exactly.

Design (per NeuronCore; batch data-parallel over 8 cores, 2 per core):
  * ALL layout work happens on host: Q^T (pre-scaled by 1/8) and K^T in
    f16, V in f32, and the mask TRANSPOSED to [k, q] in f16; output
    returns f16 and is upcast on host. The device does zero transposes
    and zero dtype-conversion DMAs (v1 burned ~33 MB/core of HBM
    traffic and a whole prep phase on mask cast+transpose).
  * Work in the transposed score orientation S_T[k, q], so the softmax
    reduction (over q) is a free-axis reduction.
  * Software pipeline at q-half granularity over 32 (batch, k-tile)
    units: three rotating [128, 1024] PSUM score tiles (6 banks) decouple
    the per-half chain QK (PE, f16) -> exp (ACT, the pacer at ~55us/core
    busy) -> PM = E*mask & row-sum D -> Vp = V/D (Pool normalize_recip)
    -> AV.
  * MIXED mask application (XJ=6): k-tiles j<6 accumulate log(mask) into
    the score PSUM via a PE identity-matmul (exp output IS the masked
    weight; one DVE TS-accum for D); tiles j>=6 use the DVE full-width
    multiply + TS-accum. This balances DVE (~40us) against spare PE
    (~55us) — DVE f16 ops run at 1-2x on real HW, not the model's 2-4x,
    and an all-DVE mask path co-paces with ACT at ~100us/pass.
  * AV with SWAPPED operand roles: stationary = PM chunk [128k, 128q],
    moving = Vp [128k, 64v] => out[q, v] accumulates directly in PSUM
    ([128, 16, 64] f32 = 2 banks, lazy-zero bank groups); no output
    transposes. AV for unit u is emitted at unit u+5 so the in-order PE
    stream never stalls on the denominator chain.
  * ~20 dummy PE transposes at kernel start ride the input-DMA wait to
    finish the 3us p-state ramp before the first real matmul.

Measured (8 cores, in-NEFF repetition differential, paired median):
~81us steady-state per pass (v1 baseline: 139us differential, 207us
harness); scale-relative absmax error 7.9e-4 vs the fp32 reference.
Ablations show the DVE is the HW pacer (~55us busy; exp on ACT hides
completely underneath), so denominators use READ-ONLY tensor_reduce
(not TS-accum, which re-writes the tensor) and the PSUM drain runs on
ACT via activation(Copy).

Hard-won HW facts (cost model/CoreSim do NOT flag these):
  * InstTensorTensorReduce (ISA op) crashes the DVE exec unit on TRN2.
  * activation(accum_out=...) is correct on HW ONLY if the accumulator
    is read >=1 unit later (the write lands after the completion sem;
    an immediate read races -> garbage), and it serializes the ACT
    pipeline (~2x per-pass cost when used on 24 exp instrs — measured
    162us vs 81us). Not worth it; reduce on DVE instead.
  * TensorScalar/free-axis TensorReduce are illegal opcodes on Pool;
    Pool is the GPSIMD DSP engine (custom ucode ops only, no PSUM).
  * DMAs issued from the ACT queue stall the exp stream badly.
  * gpsimd-queue DMAs burn ~1.3us of Pool ENGINE time each (SWDGE).
  * Inputs generated on the neuron backend (vs CPU) can contain EXACT
    mask zeros; log(0)=-inf through the identity-matmul add makes
    0*inf=NaN — the host prep clamps log(mask) to -60000 (exp still
    underflows to exactly 0). Always validate with device-generated
    inputs, not just CPU-generated ones.
"""

import os
import numpy as np

B, LQ, LK, D, DV = 16, 2048, 2048, 64, 64
NCORES = 8
BPC = B // NCORES  # batches per core
P = 128
CH = 512  # QK matmul moving chunk (one PSUM bank of fp32)
NT_Q = LQ // P  # 16
NT_K = LK // P  # 16
SCALE = 1.0 / 8.0  # 1/sqrt(64), folded into host-side Q^T prep

MAIN_REPS = int(os.environ.get("MAIN_REPS", "1"))  # repeat body (timing builds)
ABLATE = os.environ.get("ABLATE", "")  # timing-only ablations: nodve|noact|noav
# k-tiles j < XJ use the "additive" mask method: log(mask) is accumulated
# into the score PSUM by a PE identity-matmul and ACT's exp accumulates the
# softmax denominator itself — zero DVE work for those units. The rest use
# the DVE multiply path. This balances DVE (whose f16 ops run at 1-2x on
# real HW, not the cost model's 2-4x) against the PE's spare throughput.
XJ = int(os.environ.get("XJ", "6"))
DEFER_UNITS = int(os.environ.get("DEFER", "5"))  # AV emission lag (units)

_CACHED = None


def prep_core_inputs(query, key, value, mask):
    """Host-side layout prep: per-core input dicts for the device binary.

    qt: [BPC, 64, 2048] f16 = (Q/8)^T     kt: [BPC, 64, 2048] f16 = K^T
    v:  [BPC, 2048, 64] f16               mt: [2048, 2048] f16 = mask[0]^T
    """
    query = np.asarray(query, dtype=np.float32)
    key = np.asarray(key, dtype=np.float32)
    value = np.asarray(value, dtype=np.float32)
    mask = np.asarray(mask, dtype=np.float32)

    q16 = (query.transpose(0, 2, 1) * np.float32(SCALE)).astype(np.float16)
    k16 = key.transpose(0, 2, 1).astype(np.float16)
    mt = mask[0].T  # [k, q]
    m16 = mt[XJ * P :].astype(np.float16)  # multiply-path tiles (j >= XJ)
    with np.errstate(divide="ignore"):
        # additive-path tiles; clamp FINITE: mask==0 -> log = -inf would
        # hit 0*inf=NaN inside the identity-matmul add. exp(S-60000)
        # underflows to exactly 0, matching the reference's clamp.
        lm16 = np.maximum(np.log(mt[: XJ * P]), np.float32(-60000.0)).astype(
            np.float16
        )
    return [
        {
            "qt": np.ascontiguousarray(q16[c * BPC : (c + 1) * BPC]),
            "kt": np.ascontiguousarray(k16[c * BPC : (c + 1) * BPC]),
            "v": np.ascontiguousarray(value[c * BPC : (c + 1) * BPC]),
            "mt": m16,
            "lm": lm16,
        }
        for c in range(NCORES)
    ]


def _emit_av(nc, O, PM, Vp, j):
    """AV with swapped roles: stationary PM chunk [128k, 128q], moving
    Vp [128k, 64v] -> out[q, v] accumulates directly in PSUM (2 banks).

    PSUM accumulation groups are bank-granular (2 KB zero regions, 8
    chunks of [128, 64] f32 per bank): open each bank's group with
    start=True on its first chunk at j==0 (lazy-zeroes the whole bank;
    later j==0 chunks land on pending-zero bytes and replace), close it
    with stop=True on its last chunk at j==15."""
    from concourse.bass import ds

    for t in range(NT_Q):
        nc.tensor.matmul(
            O[:, t, :],
            PM[:, ds(P * t, P)],
            Vp[:],
            start=(j == 0 and t % 8 == 0),
            stop=(j == NT_K - 1 and t % 8 == 7),
            skip_group_check=True,
        )


def _emit_out(nc, work, o_d, pO, pb, ident, psum_o):
    """Evacuate the [q, v]-oriented PSUM accumulator: DVE copy to SBUF
    (f32 PSUM -> f16, host upcasts), then DMA on the sync queue — in two
    halves so the copy and the DMA pipeline."""
    import concourse.mybir as mybir
    from concourse.bass import ds

    import concourse.mybir as _mb

    out_sb = work.tile(
        [P, NT_Q, DV], mybir.dt.float16, tag="osb", bufs=2, name="out_sb"
    )
    dst = o_d[pb].rearrange("(t p) d -> p t d", p=P)
    half = NT_Q // 2
    for g in range(2):
        gs = ds(g * half, half)
        # evacuate on ACT (it has slack; DVE is the pacer)
        nc.scalar.activation(
            out_sb[:, gs, :], pO[:, gs, :],
            _mb.ActivationFunctionType.Copy,
        )
        nc.sync.dma_start(dst[:, gs, :], out_sb[:, gs, :])


def _emit_pass(nc, tc, pools, aps, dts):
    """One full pass: input DMAs + 2 batches x 16 k-tiles + output DMAs."""
    import concourse.mybir as mybir
    from concourse.bass import ds, ts

    qt_d, kt_d, v_d, mt_d, lm_d, o_d = aps
    f32, f16, AF = dts
    ALU = mybir.AluOpType
    big, psum_s, psum_o, work, small, ident, ident16 = pools
    HF0 = LQ // 2

    mT = big.tile([P, NT_K - XJ, LQ], f16, tag="mT", name="mT")
    lmT = big.tile([P, XJ, LQ], f16, tag="lmT", name="lmT")
    QT = big.tile([D, BPC, LQ], f16, tag="QT", name="QT")
    KT = big.tile([D, BPC, LK], f16, tag="KT", name="KT")
    Vn = big.tile([P, BPC, NT_K, DV], f32, tag="Vn", name="Vn")

    # ALL input DMAs go on the sync queue (hardware DGE — the gpsimd
    # queue's software DGE burns ~1.3us of Pool ENGINE time per
    # transfer), hand-ordered so each tile lands just before its first
    # use: batch 0's K/Q first (first QK), then early mask tiles (tile j
    # is consumed at ~2.2us*j), V0 (first Vp), batch 1's K/Q, and the
    # remaining mask tiles, which stay ahead of consumption from there.
    def dma_v(b):
        nc.sync.dma_start(
            Vn[:, b, :, :], v_d[b].rearrange("(t p) d -> p t d", p=P)
        )

    def dma_m(j):
        # all mask tiles on the SP hardware-DGE queue (DMAs issued from
        # the ACT queue measurably stall the exp stream — do not split)
        if j < XJ:
            nc.sync.dma_start(lmT[:, j, :], lm_d[ds(P * j, P), :])
        else:
            nc.sync.dma_start(
                mT[:, j - XJ, :], mt_d[ds(P * (j - XJ), P), :]
            )

    # just the slices the first QK touches (~150 KB), so the first score
    # matmul can issue ~1us earlier than a full-tile load would allow
    nc.sync.dma_start(KT[:, 0, ds(0, P)], kt_d[0][:, ds(0, P)])
    nc.sync.dma_start(QT[:, 0, ds(0, HF0)], qt_d[0][:, ds(0, HF0)])
    nc.sync.dma_start(KT[:, 0, ds(P, LK - P)], kt_d[0][:, ds(P, LK - P)])
    nc.sync.dma_start(QT[:, 0, ds(HF0, HF0)], qt_d[0][:, ds(HF0, HF0)])
    dma_m(0)
    dma_m(1)
    dma_m(2)
    dma_v(0)
    nc.sync.dma_start(KT[:, 1, :], kt_d[1])
    nc.sync.dma_start(QT[:, 1, :], qt_d[1])
    dma_m(3)
    dma_v(1)
    for j in range(4, NT_K):
        dma_m(j)

    # Software pipeline at q-half granularity. The two q-halves of the
    # score tile live in SEPARATE PSUM tiles (2 banks each) so dependency
    # tracking is per-half: QK for half t+1 is emitted one ACT-slot ahead
    # of ACT for half t, so the exp stream never waits on the PE.
    # Cross-engine consumers are emitted with a lag so no in-order engine
    # stream ever blocks on a slow producer:
    #   - reciprocal/Vp for unit u are emitted during unit u+1,
    #   - AV matmuls for unit u are emitted during unit u+DEFER (the
    #     denominator chain ACT->DVE->Pool->recip->Vp is ~2.5 units long
    #     and the in-order PE stream would stall on the AV Ldweights).
    # Three rotating half-score tiles (2 banks each; the swapped AV's
    # 2-bank O frees the room): QK for half t+1 only has a WAR against
    # ACT of half t-2, giving the PE a full extra ACT slot of slack.
    HF = LQ // 2
    NS = 3
    S3 = [
        psum_s.tile([P, HF], f32, tag=f"s{h}", name=f"S{h}")
        for h in range(NS)
    ]
    h0s, h1s = ds(0, HF), ds(HF, HF)
    DEFER = DEFER_UNITS

    # PE p-state warmup: ~20 dep-free dummy transposes into S3[0] (junk;
    # overwritten by the first QK) run during the input-DMA wait so the
    # 3us ramp to full clock is over by the time real matmuls issue.
    for _ in range(20):
        nc.tensor.transpose(
            S3[0][0:DV, 0:DV], ident[0:DV, 0:DV], ident[0:DV, 0:DV]
        )

    units = [(b, j) for b in range(BPC) for j in range(NT_K)]
    NU = len(units)
    ctx = {}  # u -> dict of tiles
    O_of = {}  # b -> O psum tile

    def emit_qk(t):
        u, h = t // 2, t % 2
        b, j = units[u]
        addm = j < XJ  # additive-mask unit: accumulate log(mask) on PE
        for c in range(2):
            nc.tensor.matmul(
                S3[t % NS][:, ts(c, CH)],
                KT[:, b, ds(P * j, P)],
                QT[:, b, ds(HF * h + CH * c, CH)],
                start=True,
                stop=not addm,
            )
        if addm:
            for c in range(2):
                nc.tensor.matmul(
                    S3[t % NS][:, ts(c, CH)],
                    ident16[:],
                    lmT[:, j, ds(HF * h + CH * c, CH)],
                    start=False,
                    stop=True,
                )

    emit_qk(0)
    for t in range(2 * (NU + DEFER) + 2):
        u, h = t // 2, t % 2
        if t + 1 < 2 * NU:
            emit_qk(t + 1)
        if h == 1 and u - DEFER >= 0 and u - DEFER < NU:
            ua = u - DEFER
            ca = ctx[ua]
            ba, ja = units[ua]
            if ABLATE != "noav":
                _emit_av(nc, O_of[ba], ca["PM"], ca["Vp"], ja)
            if ja == NT_K - 1:
                _emit_out(nc, work, o_d, O_of[ba], ba, ident, psum_o)
            del ctx[ua]
        if u >= NU:
            continue
        b, j = units[u]
        addm = j < XJ
        if h == 0:
            E = work.tile([P, LQ], f16, tag="e", name="E")
            ctx[u] = {"E": E}
            if not addm:
                ctx[u]["PM"] = work.tile([P, LQ], f16, tag="pm", name="PM")
            if b not in O_of:
                O_of[b] = psum_o.tile(
                    [P, NT_Q, DV], f32, tag="o", name=f"O{b}"
                )
                if ABLATE == "noav":
                    nc.vector.memset(O_of[b][:, 0, :], 0.0)
            if ABLATE != "noact":
                nc.scalar.activation(E[:, h0s], S3[t % NS][:], AF.Exp)
            else:
                nc.vector.memset(E[:, ds(0, 32)], 1.0)
        else:
            E = ctx[u]["E"]
            if ABLATE != "noact":
                nc.scalar.activation(E[:, h1s], S3[t % NS][:], AF.Exp)
            # lagged Vp for the PREVIOUS unit: one Pool-local
            # normalize_recip (Vp = V / D), so the denominator tail never
            # leaves the Pool engine's in-order stream.
            if u - 1 >= 0 and "Dsum" in ctx.get(u - 1, {}):
                cp = ctx[u - 1]
                bp, jp = units[u - 1]
                Vp = small.tile([P, DV], f16, tag="vp", name="Vp")
                nc.gpsimd.normalize_recip(
                    Vp[:], Vn[:, bp, jp, :], cp["Dsum"][:]
                )
                cp["Vp"] = Vp
            Dsum = small.tile([P, 1], f32, tag="d", name="Dsum")
            if addm:
                # additive-mask unit: the masked weights ARE exp's output;
                # one READ-ONLY reduction computes the denominator (a
                # TS-accum would re-write the full tensor — two SBUF
                # streams instead of one).
                nc.vector.tensor_reduce(
                    Dsum[:], E[:], axis=mybir.AxisListType.X, op=ALU.add
                )
                ctx[u]["PM"] = E
            elif ABLATE != "nodve":
                PM = ctx[u]["PM"]
                # DVE: full-width mask multiply, then read-only reduction
                nc.vector.tensor_tensor(
                    PM[:], E[:], mT[:, j - XJ, :], ALU.mult
                )
                nc.vector.tensor_reduce(
                    Dsum[:], PM[:], axis=mybir.AxisListType.X, op=ALU.add
                )
            else:  # timing ablation: tiny writes so tiles count as allocated
                nc.vector.memset(ctx[u]["PM"][:, ds(0, 32)], 1.0)
                nc.vector.memset(Dsum[:], 1.0)
            ctx[u]["Dsum"] = Dsum
            if u == NU - 1:  # no u+1 step will emit our Vp
                Vp = small.tile([P, DV], f16, tag="vp", name="Vp")
                nc.gpsimd.normalize_recip(Vp[:], Vn[:, b, j, :], Dsum[:])
                ctx[u]["Vp"] = Vp


def _build_module():
    import concourse.mybir as mybir
    import concourse.tile as tile
    from concourse import bacc
    from contextlib import ExitStack

    f32 = mybir.dt.float32
    f16 = mybir.dt.float16
    dts = (f32, f16, mybir.ActivationFunctionType)

    nc = bacc.Bacc("TRN2", target_bir_lowering=False, debug=False)
    qt_d = nc.dram_tensor("qt", [BPC, D, LQ], f16, kind="ExternalInput").ap()
    kt_d = nc.dram_tensor("kt", [BPC, D, LK], f16, kind="ExternalInput").ap()
    v_d = nc.dram_tensor("v", [BPC, LK, DV], f32, kind="ExternalInput").ap()
    mt_d = nc.dram_tensor(
        "mt", [LK - XJ * P, LQ], f16, kind="ExternalInput"
    ).ap()
    lm_d = nc.dram_tensor("lm", [XJ * P, LQ], f16, kind="ExternalInput").ap()
    o_d = nc.dram_tensor("o", [BPC, LQ, DV], f16, kind="ExternalOutput").ap()
    aps = (qt_d, kt_d, v_d, mt_d, lm_d, o_d)

    with tile.TileContext(nc) as tc:
        with ExitStack() as ctx:
            from concourse.masks import make_identity

            big = ctx.enter_context(tc.tile_pool(name="big", bufs=1))
            psum_s = ctx.enter_context(
                tc.tile_pool(name="psum_s", bufs=1, space="PSUM")
            )
            psum_o = ctx.enter_context(
                tc.tile_pool(name="psum_o", bufs=1, space="PSUM")
            )
            work = ctx.enter_context(tc.tile_pool(name="work", bufs=8))
            small = ctx.enter_context(tc.tile_pool(name="small", bufs=8))
            consts = ctx.enter_context(tc.tile_pool(name="consts", bufs=1))
            ident = consts.tile([P, P], f32)
            make_identity(nc, ident)
            ident16 = consts.tile([P, P], f16)
            make_identity(nc, ident16)
            pools = (big, psum_s, psum_o, work, small, ident, ident16)
            for _ in range(MAIN_REPS):
                _emit_pass(nc, tc, pools, aps, dts)

    nc.compile()
    return nc


def _get_module():
    global _CACHED
    if _CACHED is None:
        _CACHED = _build_module()
    return _CACHED


def kernel(query, key, value, mask, _trace=False):
    from concourse.bass_utils import run_bass_kernel_spmd

    nc = _get_module()
    in_maps = prep_core_inputs(query, key, value, mask)
    res = run_bass_kernel_spmd(
        nc, in_maps, core_ids=list(range(NCORES)), trace=_trace
    )
    out = np.concatenate(
        [res.results[c]["o"] for c in range(NCORES)], axis=0
    ).astype(np.float32)  # device returns f16; upcast to the contract dtype
    if _trace:
        return out, res
    return out


# revision 93
# speedup vs baseline: 2.2499x; 1.0548x over previous
"""Trainium2 Bass kernel for nn_DotProductAttention_76338748719461.

Attention with a multiplicative mask and softmax over the QUERY axis
(axis=1 of [B, Lq, Lk] scores):

    S[b,q,k]  = (Q[b,q,:] . K[b,k,:]) / 8 + max(log(mask[0,q,k]), F32_MIN)
    A         = softmax(S, axis=q)
    out[b,q,v]= sum_k A[b,q,k] * V[b,k,v]

Key identity: exp(S + log m) = exp(S) * m, so the mask is applied as a
multiply after exp — no log, no additive bias, and mask==0 handled exactly.

Design (per NeuronCore; batch data-parallel over 8 cores, 2 per core):
  * ALL layout work happens on host: Q^T (pre-scaled by 1/8) and K^T in
    f16, V in f32, and the mask TRANSPOSED to [k, q] in f16; output
    returns f16 and is upcast on host. The device does zero transposes
    and zero dtype-conversion DMAs (v1 burned ~33 MB/core of HBM
    traffic and a whole prep phase on mask cast+transpose).
  * Work in the transposed score orientation S_T[k, q], so the softmax
    reduction (over q) is a free-axis reduction.
  * Software pipeline at q-half granularity over 32 (batch, k-tile)
    units: three rotating [128, 1024] PSUM score tiles (6 banks) decouple
    the per-half chain QK (PE, f16) -> exp (ACT, the pacer at ~55us/core
    busy) -> PM = E*mask & row-sum D -> Vp = V/D (Pool normalize_recip)
    -> AV.
  * MIXED mask application (XJ=6): k-tiles j<6 accumulate log(mask) into
    the score PSUM via a PE identity-matmul (exp output IS the masked
    weight; one DVE TS-accum for D); tiles j>=6 use the DVE full-width
    multiply + TS-accum. This balances DVE (~40us) against spare PE
    (~55us) — DVE f16 ops run at 1-2x on real HW, not the model's 2-4x,
    and an all-DVE mask path co-paces with ACT at ~100us/pass.
  * AV with SWAPPED operand roles: stationary = PM chunk [128k, 128q],
    moving = Vp [128k, 64v] => out[q, v] accumulates directly in PSUM
    ([128, 16, 64] f32 = 2 banks, lazy-zero bank groups); no output
    transposes. AV for unit u is emitted at unit u+5 so the in-order PE
    stream never stalls on the denominator chain.
  * ~20 dummy PE transposes at kernel start ride the input-DMA wait to
    finish the 3us p-state ramp before the first real matmul.

Measured (8 cores, in-NEFF repetition differential, paired median):
~81us steady-state per pass (v1 baseline: 139us differential, 207us
harness); scale-relative absmax error 7.9e-4 vs the fp32 reference.
HW ablation decomposition (ABLATE=..., REPS=33 paired medians):
full ~81-87us; noact 82.5 (exp FULLY hidden, marginal ~0); noav 76
(AV + its 512 ldweights marginal ~5-10us); nodve 64 (DVE mask/accum
marginal ~19-23us -> DVE is the pacer). So denominators use READ-ONLY
tensor_reduce (not TS-accum, which re-writes the tensor) and the PSUM
drain runs on ACT via activation(Copy). The ~55-60us base under all
ablations = QK matmuls + DMA stream + scheduling skeleton.

Hard-won HW facts (cost model/CoreSim do NOT flag these):
  * InstTensorTensorReduce (ISA op) crashes the DVE exec unit on TRN2.
  * activation(accum_out=...) is correct on HW ONLY if the accumulator
    is read >=1 unit later (the write lands after the completion sem;
    an immediate read races -> garbage), and it serializes the ACT
    pipeline (~2x per-pass cost when used on 24 exp instrs — measured
    162us vs 81us). Not worth it; reduce on DVE instead.
  * TensorScalar/free-axis TensorReduce are illegal opcodes on Pool;
    Pool is the GPSIMD DSP engine (custom ucode ops only, no PSUM).
  * DMAs issued from the ACT queue stall the exp stream badly.
  * gpsimd-queue DMAs burn ~1.3us of Pool ENGINE time each (SWDGE).
  * Inputs generated on the neuron backend (vs CPU) can contain EXACT
    mask zeros; log(0)=-inf through the identity-matmul add makes
    0*inf=NaN — the host prep clamps log(mask) to -60000 (exp still
    underflows to exactly 0). Always validate with device-generated
    inputs, not just CPU-generated ones.
"""

import os
import numpy as np

B, LQ, LK, D, DV = 16, 2048, 2048, 64, 64
NCORES = 8
BPC = B // NCORES  # batches per core
P = 128
CH = 512  # QK matmul moving chunk (one PSUM bank of fp32)
NT_Q = LQ // P  # 16
NT_K = LK // P  # 16
SCALE = 1.0 / 8.0  # 1/sqrt(64), folded into host-side Q^T prep

MAIN_REPS = int(os.environ.get("MAIN_REPS", "1"))  # repeat body (timing builds)
ABLATE = os.environ.get("ABLATE", "")  # timing-only ablations: nodve|noact|noav
# k-tiles j < XJ use the "additive" mask method: log(mask) is accumulated
# into the score PSUM by a PE identity-matmul and ACT's exp accumulates the
# softmax denominator itself — zero DVE work for those units. The rest use
# the DVE multiply path. This balances DVE (whose f16 ops run at 1-2x on
# real HW, not the cost model's 2-4x) against the PE's spare throughput.
XJ = int(os.environ.get("XJ", "6"))
DEFER_UNITS = int(os.environ.get("DEFER", "5"))  # AV emission lag (units)

_CACHED = None


def prep_core_inputs(query, key, value, mask):
    """Host-side layout prep: per-core input dicts for the device binary.

    qt: [BPC, 64, 2048] f16 = (Q/8)^T     kt: [BPC, 64, 2048] f16 = K^T
    v:  [BPC, 2048, 64] f16               mt: [2048, 2048] f16 = mask[0]^T
    """
    query = np.asarray(query, dtype=np.float32)
    key = np.asarray(key, dtype=np.float32)
    value = np.asarray(value, dtype=np.float32)
    mask = np.asarray(mask, dtype=np.float32)

    q16 = (query.transpose(0, 2, 1) * np.float32(SCALE)).astype(np.float16)
    k16 = key.transpose(0, 2, 1).astype(np.float16)
    mt = mask[0].T  # [k, q]
    m16 = mt[XJ * P :].astype(np.float16)  # multiply-path tiles (j >= XJ)
    with np.errstate(divide="ignore"):
        # additive-path tiles; clamp FINITE: mask==0 -> log = -inf would
        # hit 0*inf=NaN inside the identity-matmul add. exp(S-60000)
        # underflows to exactly 0, matching the reference's clamp.
        lm16 = np.maximum(np.log(mt[: XJ * P]), np.float32(-60000.0)).astype(
            np.float16
        )
    return [
        {
            "qt": np.ascontiguousarray(q16[c * BPC : (c + 1) * BPC]),
            "kt": np.ascontiguousarray(k16[c * BPC : (c + 1) * BPC]),
            "v": np.ascontiguousarray(value[c * BPC : (c + 1) * BPC]),
            "mt": m16,
            "lm": lm16,
        }
        for c in range(NCORES)
    ]


def _emit_av(nc, O, PM, Vp, j):
    """AV with swapped roles: stationary PM chunk [128k, 128q], moving
    Vp [128k, 64v] -> out[q, v] accumulates directly in PSUM (2 banks).

    PSUM accumulation groups are bank-granular (2 KB zero regions, 8
    chunks of [128, 64] f32 per bank): open each bank's group with
    start=True on its first chunk at j==0 (lazy-zeroes the whole bank;
    later j==0 chunks land on pending-zero bytes and replace), close it
    with stop=True on its last chunk at j==15."""
    from concourse.bass import ds

    for t in range(NT_Q):
        nc.tensor.matmul(
            O[:, t, :],
            PM[:, ds(P * t, P)],
            Vp[:],
            start=(j == 0 and t % 8 == 0),
            stop=(j == NT_K - 1 and t % 8 == 7),
            skip_group_check=True,
        )


def _emit_out(nc, work, o_d, pO, pb, ident, psum_o):
    """Evacuate the [q, v]-oriented PSUM accumulator: DVE copy to SBUF
    (f32 PSUM -> f16, host upcasts), then DMA on the sync queue — in two
    halves so the copy and the DMA pipeline."""
    import concourse.mybir as mybir
    from concourse.bass import ds

    import concourse.mybir as _mb

    out_sb = work.tile(
        [P, NT_Q, DV], mybir.dt.float16, tag="osb", bufs=2, name="out_sb"
    )
    dst = o_d[pb].rearrange("(t p) d -> p t d", p=P)
    half = NT_Q // 2
    for g in range(2):
        gs = ds(g * half, half)
        # evacuate on ACT (it has slack; DVE is the pacer)
        nc.scalar.activation(
            out_sb[:, gs, :], pO[:, gs, :],
            _mb.ActivationFunctionType.Copy,
        )
        nc.sync.dma_start(dst[:, gs, :], out_sb[:, gs, :])


def _emit_pass(nc, tc, pools, aps, dts):
    """One full pass: input DMAs + 2 batches x 16 k-tiles + output DMAs."""
    import concourse.mybir as mybir
    from concourse.bass import ds, ts

    qt_d, kt_d, v_d, mt_d, lm_d, o_d = aps
    f32, f16, AF = dts
    ALU = mybir.AluOpType
    big, psum_s, psum_o, work, small, ident, ident16 = pools
    HF0 = LQ // 2

    mT = big.tile([P, NT_K - XJ, LQ], f16, tag="mT", name="mT")
    lmT = big.tile([P, XJ, LQ], f16, tag="lmT", name="lmT")
    QT = big.tile([D, BPC, LQ], f16, tag="QT", name="QT")
    KT = big.tile([D, BPC, LK], f16, tag="KT", name="KT")
    Vn = big.tile([P, BPC, NT_K, DV], f32, tag="Vn", name="Vn")

    # ALL input DMAs go on the sync queue (hardware DGE — the gpsimd
    # queue's software DGE burns ~1.3us of Pool ENGINE time per
    # transfer), hand-ordered so each tile lands just before its first
    # use: batch 0's K/Q first (first QK), then early mask tiles (tile j
    # is consumed at ~2.2us*j), V0 (first Vp), batch 1's K/Q, and the
    # remaining mask tiles, which stay ahead of consumption from there.
    def dma_v(b):
        nc.sync.dma_start(
            Vn[:, b, :, :], v_d[b].rearrange("(t p) d -> p t d", p=P)
        )

    def dma_m(j):
        # all mask tiles on the SP hardware-DGE queue (DMAs issued from
        # the ACT queue measurably stall the exp stream — do not split)
        if j < XJ:
            nc.sync.dma_start(lmT[:, j, :], lm_d[ds(P * j, P), :])
        else:
            nc.sync.dma_start(
                mT[:, j - XJ, :], mt_d[ds(P * (j - XJ), P), :]
            )

    # just the slices the first QK touches (~150 KB), so the first score
    # matmul can issue ~1us earlier than a full-tile load would allow
    nc.sync.dma_start(KT[:, 0, ds(0, P)], kt_d[0][:, ds(0, P)])
    nc.sync.dma_start(QT[:, 0, ds(0, HF0)], qt_d[0][:, ds(0, HF0)])
    nc.sync.dma_start(KT[:, 0, ds(P, LK - P)], kt_d[0][:, ds(P, LK - P)])
    nc.sync.dma_start(QT[:, 0, ds(HF0, HF0)], qt_d[0][:, ds(HF0, HF0)])
    dma_m(0)
    dma_m(1)
    dma_m(2)
    dma_v(0)
    nc.sync.dma_start(KT[:, 1, :], kt_d[1])
    nc.sync.dma_start(QT[:, 1, :], qt_d[1])
    dma_m(3)
    dma_v(1)
    for j in range(4, NT_K):
        dma_m(j)

    # Software pipeline at q-half granularity. The two q-halves of the
    # score tile live in SEPARATE PSUM tiles (2 banks each) so dependency
    # tracking is per-half: QK for half t+1 is emitted one ACT-slot ahead
    # of ACT for half t, so the exp stream never waits on the PE.
    # Cross-engine consumers are emitted with a lag so no in-order engine
    # stream ever blocks on a slow producer:
    #   - reciprocal/Vp for unit u are emitted during unit u+1,
    #   - AV matmuls for unit u are emitted during unit u+DEFER (the
    #     denominator chain ACT->DVE->Pool->recip->Vp is ~2.5 units long
    #     and the in-order PE stream would stall on the AV Ldweights).
    # Three rotating half-score tiles (2 banks each; the swapped AV's
    # 2-bank O frees the room): QK for half t+1 only has a WAR against
    # ACT of half t-2, giving the PE a full extra ACT slot of slack.
    HF = LQ // 2
    NS = 3
    S3 = [
        psum_s.tile([P, HF], f32, tag=f"s{h}", name=f"S{h}")
        for h in range(NS)
    ]
    h0s, h1s = ds(0, HF), ds(HF, HF)
    DEFER = DEFER_UNITS

    # PE p-state warmup: ~20 dep-free dummy transposes into S3[0] (junk;
    # overwritten by the first QK) run during the input-DMA wait so the
    # 3us ramp to full clock is over by the time real matmuls issue.
    for _ in range(20):
        nc.tensor.transpose(
            S3[0][0:DV, 0:DV], ident[0:DV, 0:DV], ident[0:DV, 0:DV]
        )

    units = [(b, j) for b in range(BPC) for j in range(NT_K)]
    NU = len(units)
    ctx = {}  # u -> dict of tiles
    O_of = {}  # b -> O psum tile

    def emit_qk(t):
        u, h = t // 2, t % 2
        b, j = units[u]
        addm = j < XJ  # additive-mask unit: accumulate log(mask) on PE
        for c in range(2):
            nc.tensor.matmul(
                S3[t % NS][:, ts(c, CH)],
                KT[:, b, ds(P * j, P)],
                QT[:, b, ds(HF * h + CH * c, CH)],
                start=True,
                stop=not addm,
            )
        if addm:
            for c in range(2):
                nc.tensor.matmul(
                    S3[t % NS][:, ts(c, CH)],
                    ident16[:],
                    lmT[:, j, ds(HF * h + CH * c, CH)],
                    start=False,
                    stop=True,
                )

    emit_qk(0)
    for t in range(2 * (NU + DEFER) + 2):
        u, h = t // 2, t % 2
        if t + 1 < 2 * NU:
            emit_qk(t + 1)
        if h == 1 and u - DEFER >= 0 and u - DEFER < NU:
            ua = u - DEFER
            ca = ctx[ua]
            ba, ja = units[ua]
            if ABLATE != "noav":
                _emit_av(nc, O_of[ba], ca["PM"], ca["Vp"], ja)
            if ja == NT_K - 1:
                _emit_out(nc, work, o_d, O_of[ba], ba, ident, psum_o)
            del ctx[ua]
        if u >= NU:
            continue
        b, j = units[u]
        addm = j < XJ
        if h == 0:
            E = work.tile([P, LQ], f16, tag="e", name="E")
            ctx[u] = {"E": E}
            if not addm:
                ctx[u]["PM"] = work.tile([P, LQ], f16, tag="pm", name="PM")
            if b not in O_of:
                O_of[b] = psum_o.tile(
                    [P, NT_Q, DV], f32, tag="o", name=f"O{b}"
                )
                if ABLATE == "noav":
                    nc.vector.memset(O_of[b][:, 0, :], 0.0)
            if ABLATE != "noact":
                nc.scalar.activation(E[:, h0s], S3[t % NS][:], AF.Exp)
            else:
                nc.vector.memset(E[:, ds(0, 32)], 1.0)
        else:
            E = ctx[u]["E"]
            if ABLATE != "noact":
                nc.scalar.activation(E[:, h1s], S3[t % NS][:], AF.Exp)
            # lagged Vp for the PREVIOUS unit: one Pool-local
            # normalize_recip (Vp = V / D), so the denominator tail never
            # leaves the Pool engine's in-order stream.
            if u - 1 >= 0 and "Dsum" in ctx.get(u - 1, {}):
                cp = ctx[u - 1]
                bp, jp = units[u - 1]
                Vp = small.tile([P, DV], f16, tag="vp", name="Vp")
                nc.gpsimd.normalize_recip(
                    Vp[:], Vn[:, bp, jp, :], cp["Dsum"][:]
                )
                cp["Vp"] = Vp
            Dsum = small.tile([P, 1], f32, tag="d", name="Dsum")
            if addm:
                # additive-mask unit: the masked weights ARE exp's output;
                # one READ-ONLY reduction computes the denominator (a
                # TS-accum would re-write the full tensor — two SBUF
                # streams instead of one).
                nc.vector.tensor_reduce(
                    Dsum[:], E[:], axis=mybir.AxisListType.X, op=ALU.add
                )
                ctx[u]["PM"] = E
            elif ABLATE != "nodve":
                PM = ctx[u]["PM"]
                # DVE: full-width mask multiply, then read-only reduction
                nc.vector.tensor_tensor(
                    PM[:], E[:], mT[:, j - XJ, :], ALU.mult
                )
                nc.vector.tensor_reduce(
                    Dsum[:], PM[:], axis=mybir.AxisListType.X, op=ALU.add
                )
            else:  # timing ablation: tiny writes so tiles count as allocated
                nc.vector.memset(ctx[u]["PM"][:, ds(0, 32)], 1.0)
                nc.vector.memset(Dsum[:], 1.0)
            ctx[u]["Dsum"] = Dsum
            if u == NU - 1:  # no u+1 step will emit our Vp
                Vp = small.tile([P, DV], f16, tag="vp", name="Vp")
                nc.gpsimd.normalize_recip(Vp[:], Vn[:, b, j, :], Dsum[:])
                ctx[u]["Vp"] = Vp


def _build_module():
    import concourse.mybir as mybir
    import concourse.tile as tile
    from concourse import bacc
    from contextlib import ExitStack

    f32 = mybir.dt.float32
    f16 = mybir.dt.float16
    dts = (f32, f16, mybir.ActivationFunctionType)

    nc = bacc.Bacc("TRN2", target_bir_lowering=False, debug=False)
    qt_d = nc.dram_tensor("qt", [BPC, D, LQ], f16, kind="ExternalInput").ap()
    kt_d = nc.dram_tensor("kt", [BPC, D, LK], f16, kind="ExternalInput").ap()
    v_d = nc.dram_tensor("v", [BPC, LK, DV], f32, kind="ExternalInput").ap()
    mt_d = nc.dram_tensor(
        "mt", [LK - XJ * P, LQ], f16, kind="ExternalInput"
    ).ap()
    lm_d = nc.dram_tensor("lm", [XJ * P, LQ], f16, kind="ExternalInput").ap()
    o_d = nc.dram_tensor("o", [BPC, LQ, DV], f16, kind="ExternalOutput").ap()
    aps = (qt_d, kt_d, v_d, mt_d, lm_d, o_d)

    with tile.TileContext(nc) as tc:
        with ExitStack() as ctx:
            from concourse.masks import make_identity

            big = ctx.enter_context(tc.tile_pool(name="big", bufs=1))
            psum_s = ctx.enter_context(
                tc.tile_pool(name="psum_s", bufs=1, space="PSUM")
            )
            psum_o = ctx.enter_context(
                tc.tile_pool(name="psum_o", bufs=1, space="PSUM")
            )
            work = ctx.enter_context(tc.tile_pool(name="work", bufs=8))
            small = ctx.enter_context(tc.tile_pool(name="small", bufs=8))
            consts = ctx.enter_context(tc.tile_pool(name="consts", bufs=1))
            ident = consts.tile([P, P], f32)
            make_identity(nc, ident)
            ident16 = consts.tile([P, P], f16)
            make_identity(nc, ident16)
            pools = (big, psum_s, psum_o, work, small, ident, ident16)
            for _ in range(MAIN_REPS):
                _emit_pass(nc, tc, pools, aps, dts)

    nc.compile()
    return nc


def _get_module():
    global _CACHED
    if _CACHED is None:
        _CACHED = _build_module()
    return _CACHED


def kernel(query, key, value, mask, _trace=False):
    from concourse.bass_utils import run_bass_kernel_spmd

    nc = _get_module()
    in_maps = prep_core_inputs(query, key, value, mask)
    res = run_bass_kernel_spmd(
        nc, in_maps, core_ids=list(range(NCORES)), trace=_trace
    )
    out = np.concatenate(
        [res.results[c]["o"] for c in range(NCORES)], axis=0
    ).astype(np.float32)  # device returns f16; upcast to the contract dtype
    if _trace:
        return out, res
    return out
